# revision 79
# baseline (speedup 1.0000x reference)
"""NetVLAD (vq_codebook) Trainium2 Bass kernel, 8-way spatially sharded.

Math (validated vs reference to ~1.6e-3 rel, tolerance 2e-2):
  xn = x / ||x||_C per location (HOST); logits per l-tile directly in
  [L,K] layout (stationary x-tile, moving conv weights, fp8 both);
  e = exp(logits/64) f32; top-2 keep via per-tile Max8 ([:,1] = 2nd max);
  cnt = 3x3 box-sum of keep as banded shift-matrix matmuls on the PE in
  fp8 DoubleRow mode (pairs of shift groups, overlapping rhs APs, zero
  guard tiles — no transposes, no unfold/fold); w2 = e * mask^2/sumexp *
  cnt in fp8; VLAD partials [K,C+1] = w2.T @ [xn*mask^2*64 | mask^2*64]
  in fp8 DoubleRow (two l-tiles per matmul), all-reduced on host.
  The border mask^4 is split mask^2 * mask^2 across w2 and the xlcn
  stream so both stay inside fp8 e4m3 range; all uniform scale factors
  (64, 1/max(mask^2)) cancel in the final L2 normalizations.

All HBM streams are fp8 (x ~5.2 MB/core total) packed host-side so each
SBUF partition is one contiguous DMA run; a 6-stage chunk pipeline
overlaps DMA / PE logits / Scalar exp / DVE top-2 / PE cnt / Pool ee /
DVE w2 / PE VLAD, with dummy-matmul sync absorbers keeping every
instruction at <= 1 sync wait (walrus codegen limit).

Sharding: H=192 rows split 8 ways (24 rows/core + 1 halo row each side).
Everything local per core; [K,C+1] partials reduced on host.
"""
import os
import sys

sys.path.insert(0, "/opt/trn_rl_repo")
os.environ.setdefault("MYCRO_LOCAL_CACHE", "1")

import numpy as np

C, H, W, K = 512, 192, 192, 64
M = 8                      # cores
RPC = H // M               # 24 rows per core
Ls = (RPC + 2) * W         # 4992 slab locations (incl. 1 halo row each side)
NT = Ls // 128             # 39 l-tiles
CT = C // 128              # 4 c-tiles
CH = 8                     # max tiles per pipeline chunk
# small final chunks shorten the serial dependency tail after the last
# DMA wave lands
CHUNK_SIZES = [6, 6, 6, 6, 6, 6, 3]
assert sum(CHUNK_SIZES) == NT
CHUNKS = []
_o = 0
for _s in CHUNK_SIZES:
    CHUNKS.append((_o, _o + _s))
    _o += _s

TRACE = False              # set by test.py for profiling runs
_CACHE = {}


def _build_nc():
    import concourse.bass as bass
    import concourse.tile as tile
    from concourse import mybir

    f32 = mybir.dt.float32
    bf16 = mybir.dt.bfloat16
    f8 = mybir.dt.float8e4
    AF = mybir.ActivationFunctionType
    OP = mybir.AluOpType
    AX = mybir.AxisListType

    nc = bass.Bass()
    # host-packed layouts: one contiguous run per partition -> wide DMA
    # descriptors (the [Ls, C+1] row layout was descriptor-bound in fp8)
    xcl = nc.dram_tensor("xcl", [128, CT * Ls], f8, kind="ExternalInput")
    xlcn = nc.dram_tensor("xlcn", [128, NT * (C + 1)], f8,
                          kind="ExternalInput")
    c8 = nc.dram_tensor("c8", [128, CT * K + 6 * 128], f8,
                        kind="ExternalInput")
    m4 = nc.dram_tensor("m4", [128, NT], f32, kind="ExternalInput")
    y = nc.dram_tensor("y", [K, C + 1], f32, kind="ExternalOutput")

    with tile.TileContext(nc) as tc:
        with tc.tile_pool(name="big", bufs=1) as big:
            xcl_sb = big.tile([128, CT * Ls], f8, tag="xcl")
            xlcn_sb = big.tile([128, NT * (C + 1)], f8, tag="xlcn")
            c8_sb = big.tile([128, CT * K + 6 * 128], f8, tag="c8")
            cwt_sb = c8_sb[:, 0:CT * K]
            shm_sb = c8_sb[:, CT * K:]
            m4_sb = big.tile([128, NT], f32, tag="m4")
            expb = big.tile([128, NT * K], f32, tag="expb")
            eeb = big.tile([128, NT * K], f32, tag="eeb")
            # 2 zero guard tiles on each side let the cnt shift matmuls run
            # full chunk width with uniform start/stop accumulation groups;
            # keep/w2 are fp8 so cnt and VLAD matmuls run in DoubleRow mode
            keeplk = big.tile([128, (NT + 5) * K], f8, tag="keeplk")
            w2b = big.tile([128, NT * K], f8, tag="w2b")
            mx8 = big.tile([128, NT * 8], f32, tag="mx8")
            sume = big.tile([128, NT], f32, tag="sume")
            isume = big.tile([128, NT], f32, tag="isume")
            isume2 = big.tile([128, NT], f32, tag="isume2")
            cnt_sb = big.tile([128, 2 * CH * K], f32, tag="cntsb")
            vl_sb = big.tile([K, C + 1], f32, tag="vl")
            zbias = big.tile([128, 1], f32, tag="zbias")
            zscr = big.tile([128, 1], f32, tag="zscr")
            nc.vector.memset(zbias[:], 0.0)
            # scalar-engine touch: absorbs the memset completion so the first
            # exp activation carries only its PE wait
            nc.scalar.copy(zscr[:], zbias[:])

            # input DMAs: small first, then xcl chunks interleaved with the
            # xlcn waves (lag 2) so VLAD waves can start mid-kernel
            xc3 = xcl[:].rearrange("p (ct l) -> p ct l", l=Ls)
            xs3 = xcl_sb[:].rearrange("p (ct l) -> p ct l", l=Ls)

            # Each dma_start costs ~0.65us of SP sequencer time, so waves are
            # coarse (11 issues total) and ordered by need: first xcl wave,
            # consts, rest of xcl (compute-gating), then xlcn.
            # (Issuing from other sequencers regressed: the Tile scheduler
            # interleaves those issues with that engine's compute work.)
            XCLW = list(CHUNKS)
            XLCNW = list(CHUNKS)

            def dma_xcl_w(t0, t1):
                nc.sync.dma_start(
                    xs3[:, :, t0 * 128:t1 * 128],
                    xc3[:, :, t0 * 128:t1 * 128],
                )

            def dma_xlcn_w(t0, t1):
                nc.sync.dma_start(
                    xlcn_sb[:, t0 * (C + 1):t1 * (C + 1)],
                    xlcn[:, t0 * (C + 1):t1 * (C + 1)],
                )

            dma_xcl_w(*XCLW[0])
            nc.sync.dma_start(c8_sb[:], c8[:])
            nc.sync.dma_start(m4_sb[:], m4[:])
            for w in XCLW[1:]:
                dma_xcl_w(*w)
            for w in XLCNW:
                dma_xlcn_w(*w)

            e3f = expb[:].rearrange("p (t k) -> p t k", k=K)
            ee3f = eeb[:].rearrange("p (t k) -> p t k", k=K)
            k3f = keeplk[:].rearrange("p (t k) -> p t k", k=K)  # guard-offset
            w23f = w2b[:].rearrange("p (t k) -> p t k", k=K)
            mx3f = mx8[:].rearrange("p (t e) -> p t e", e=8)
            # Pool touch: absorbs the m4 DMA completion so the per-chunk
            # mask multiplies carry only their DVE(recip) wait
            nc.gpsimd.tensor_copy(isume2[:, 0:1], m4_sb[:, 0:1])
            nc.vector.memset(keeplk[:, 0:2 * K], 0.0)
            nc.vector.memset(keeplk[:, (NT + 2) * K:(NT + 5) * K], 0.0)

            with tc.tile_pool(name="pp", bufs=1, space="PSUM") as pp:
                pv0 = pp.tile([K, C], f32, tag="pv0", bufs=1)
                pv1 = pp.tile([K, 1], f32, tag="pv1", bufs=1)
                pcnts = {}

                # two dummies absorb the c8 DMA wait (cwt+shm views); a long
                # HAM warm-up burst is not worth delaying logits(0) — the
                # DVE chain that paces the kernel starts after exp(chunk 0)
                for i in range(2):
                    src = shm_sb if i == 1 else cwt_sb
                    dmy = pp.tile([128, K], f32, tag="dmy", bufs=2)
                    nc.tensor.matmul(dmy[0:64, 0:64], lhsT=src[:, 0:64],
                                     rhs=src[:, 0:64], start=True, stop=True)

                def emit_logits(ci):
                    t0, t1 = CHUNKS[ci]
                    # dummy absorbs this chunk's xcl DMA completion so real
                    # matmuls carry only the psum-rotation wait
                    dmy = pp.tile([128, K], f32, tag="dmy", bufs=2)
                    nc.tensor.matmul(
                        dmy[0:64, 0:64],
                        lhsT=xcl_sb[:, t0 * 128:t0 * 128 + 64],
                        rhs=xcl_sb[:, t0 * 128:t0 * 128 + 64],
                        start=True, stop=True)
                    plog = pp.tile([128, CH * K], f32, tag="plog", bufs=2)
                    for t in range(t0, t1):
                        for ct in range(CT):
                            nc.tensor.matmul(
                                plog[:, (t - t0) * K:(t - t0 + 1) * K],
                                lhsT=xcl_sb[:, ct * Ls + t * 128:
                                            ct * Ls + (t + 1) * 128],
                                rhs=cwt_sb[:, ct * K:(ct + 1) * K],
                                start=(ct == 0), stop=(ct == CT - 1))
                    return plog

                def emit_exp(ci, plog):
                    # scale 1/64 undoes the fp8-range prescale of xcl
                    t0, t1 = CHUNKS[ci]
                    nc.scalar.activation(
                        expb[:, t0 * K:t1 * K], plog[:, 0:(t1 - t0) * K],
                        AF.Exp, bias=zbias[:, 0:1], scale=1.0 / 64.0)

                def emit_dve(ci):
                    t0, t1 = CHUNKS[ci]
                    ch = t1 - t0
                    e3 = e3f[:, t0:t1]
                    k3 = k3f[:, t0:t1]
                    # keep (gating the PE cnt matmuls) first: top-8 per tile
                    # ([:,0]=max, [:,1]=second max), then the compare
                    for t in range(t0, t1):
                        nc.vector.max(mx8[:, t * 8:(t + 1) * 8],
                                      expb[:, t * K:(t + 1) * K])
                    m2bc = mx3f[:, t0:t1, 1:2].broadcast_to([128, ch, K])
                    nc.vector.tensor_tensor(k3f[:, t0 + 2:t1 + 2], e3, m2bc,
                                            op=OP.is_ge)
                    # softmax normalizer (gates only the Pool ee multiply)
                    nc.vector.tensor_reduce(sume[:, t0:t1], e3, axis=AX.X,
                                            op=OP.add)
                    nc.vector.reciprocal(isume[:, t0:t1], sume[:, t0:t1])

                def emit_pool_ee(ci):
                    # the mask^2 fold runs on Pool too (keeps it off the
                    # pacing Vector engine); isume2 is Pool-local
                    t0, t1 = CHUNKS[ci]
                    ch = t1 - t0
                    nc.gpsimd.tensor_tensor(isume2[:, t0:t1],
                                            isume[:, t0:t1],
                                            m4_sb[:, t0:t1], op=OP.mult)
                    ibc = isume2[:, t0:t1][:, :, None].broadcast_to(
                        [128, ch, K])
                    nc.gpsimd.tensor_tensor(ee3f[:, t0:t1], e3f[:, t0:t1],
                                            ibc, op=OP.mult)

                def emit_cnt(ci):
                    # full-chunk-width DoubleRow matmuls: each contracts a
                    # pair of shift groups (j, j+1); the 6th shift matrix is
                    # zero. Zero guard tiles cover out-of-slab sources. The
                    # pair rhs is an overlapping AP (same keep data shifted
                    # by one tile).
                    t0, t1 = CHUNKS[ci]
                    w = (t1 - t0) * K
                    pcnt = pp.tile([128, CH * K], f32, tag="pcnt", bufs=2)
                    pcnts[ci] = pcnt
                    for pi, j in enumerate((-2, 0, 2)):
                        lhsT = shm_sb[:, (j + 2) * 128:(j + 4) * 128]\
                            .rearrange("p (two i) -> p two i", two=2)
                        base = keeplk[:, (t0 + j + 2) * K:(t1 + j + 2) * K]
                        rhs = type(base)(base.tensor, base.offset,
                                         [list(base.ap[0]), [K, 2], [1, w]])
                        nc.tensor.matmul(
                            pcnt[:, 0:w], lhsT=lhsT, rhs=rhs,
                            perf_mode=mybir.MatmulPerfMode.DoubleRow,
                            start=(pi == 0), stop=(pi == 2))

                def emit_pool_w2(ci):
                    # GPSIMD cannot read PSUM, so the idle Scalar engine
                    # stages cnt into SBUF and Pool does the multiply --
                    # keeping the whole w2 path off the pacing Vector engine
                    t0, t1 = CHUNKS[ci]
                    w = (t1 - t0) * K
                    half = (ci % 2) * CH * K
                    if ci >= 2:
                        # touch absorbs Pool(w2(ci-2)) so the staging copy
                        # carries only its PE(pcnt) wait (buffer-half WAR)
                        p0 = CHUNKS[ci - 2][0]
                        nc.scalar.copy(cnt_sb[:, half:half + 1],
                                       w2b[:, p0 * K:p0 * K + 1])
                    nc.scalar.copy(cnt_sb[:, half:half + w],
                                   pcnts[ci][:, 0:w])
                    pc3 = cnt_sb[:, half:half + w].rearrange(
                        "p (t k) -> p t k", k=K)
                    nc.gpsimd.tensor_tensor(w23f[:, t0:t1], ee3f[:, t0:t1],
                                            pc3, op=OP.mult)

                def emit_vlad(ci):
                    t0, t1 = CHUNKS[ci]
                    # dummy absorbs the xlcn wave DMA completion
                    dmy = pp.tile([128, K], f32, tag="dmy", bufs=2)
                    nc.tensor.matmul(
                        dmy[0:64, 0:64],
                        lhsT=xlcn_sb[:, t0 * (C + 1):t0 * (C + 1) + 64],
                        rhs=xlcn_sb[:, t0 * (C + 1):t0 * (C + 1) + 64],
                        start=True, stop=True)
                    xl3 = xlcn_sb[:].rearrange("p (t c) -> p t c", c=C + 1)
                    t = t0
                    while t < t1:
                        if t + 1 < t1:   # DoubleRow pair of l-tiles
                            lt = w2b[:, t * K:(t + 2) * K].rearrange(
                                "p (two k) -> p two k", two=2)
                            nc.tensor.matmul(
                                pv0[:], lhsT=lt, rhs=xl3[:, t:t + 2, 0:C],
                                perf_mode=mybir.MatmulPerfMode.DoubleRow,
                                start=(t == 0), stop=(t + 2 == NT))
                            nc.tensor.matmul(
                                pv1[:], lhsT=lt, rhs=xl3[:, t:t + 2, C:C + 1],
                                perf_mode=mybir.MatmulPerfMode.DoubleRow,
                                start=(t == 0), stop=(t + 2 == NT))
                            t += 2
                        else:            # odd tail tile
                            lt = w2b[:, t * K:(t + 1) * K]
                            nc.tensor.matmul(
                                pv0[:], lhsT=lt, rhs=xl3[:, t, 0:C],
                                start=(t == 0), stop=(t + 1 == NT))
                            nc.tensor.matmul(
                                pv1[:], lhsT=lt, rhs=xl3[:, t, C:C + 1],
                                start=(t == 0), stop=(t + 1 == NT))
                            t += 1

                # VLAD runs with lag 2 and is emitted BEFORE cnt: its inputs
                # (w2 of two chunks back + an old xlcn wave) are long ready,
                # so it fills the PE stall while the DVE finishes keep(ci),
                # keeping the PE busy (and its HAM clock warm)
                for ci in range(len(CHUNKS)):
                    plog = emit_logits(ci)
                    emit_exp(ci, plog)
                    emit_dve(ci)
                    emit_pool_ee(ci)
                    if ci >= 2:
                        emit_vlad(ci - 2)
                    if ci >= 1:
                        emit_cnt(ci - 1)
                        emit_pool_w2(ci - 1)
                last = len(CHUNKS) - 1
                emit_vlad(last - 1)
                emit_cnt(last)
                emit_pool_w2(last)
                emit_vlad(last)

                nc.scalar.copy(vl_sb[:, 0:C], pv0[:])
                nc.scalar.copy(vl_sb[:, C:C + 1], pv1[:])
                nc.sync.dma_start(y[:], vl_sb[:])
    _prune_waits(nc)
    return nc


def _prune_waits(nc):
    """Drop semaphore waits that are transitively implied by another wait on
    the same instruction (see kernel_baseline.py for the full rationale)."""
    insts = [ins for bb in nc.main_func.blocks for ins in bb.instructions]
    proc_events = {}
    waits_of = {}
    carried = {}   # engine -> waits of non-updating instrs (e.g. Ldweights)
    for ins in insts:
        si = getattr(ins, "sync_info", None)
        if si is None:
            continue
        ow = list(si.on_wait or [])
        waits_of[id(ins)] = [(w.ant_name, w.wait_value) for w in ow]
        ups = [u for u in (si.on_update or [])
               if getattr(u, "update_mode", None) in ("sem-inc", "sem-add-imm")]
        eng = getattr(ins, "engine", None)
        if not ups:
            # a waiting-but-not-updating instruction (Ldweights): its waits
            # are guaranteed held once the NEXT updating instruction on the
            # same engine ticks (in-order issue; LDW completes before its MM)
            if ow and eng is not None:
                carried.setdefault(eng, []).extend(waits_of[id(ins)])
            continue
        if eng in carried and carried[eng]:
            waits_of[id(ins)] = waits_of[id(ins)] + carried.pop(eng)
        for u in ups:
            lst = proc_events.setdefault(u.ant_name, [])
            prev = lst[-1][0] if lst else 0
            lst.append((prev + (u.update_value or 1), ins))

    import bisect

    def prefix_index(sem, v):
        lst = proc_events.get(sem)
        if not lst:
            return None
        ticks = [t for t, _ in lst]
        i = bisect.bisect_left(ticks, v)
        return i if i < len(lst) else None

    memo = {}

    def holds(sem, v, depth=0):
        if depth > 6:
            return {}
        i = prefix_index(sem, v)
        if i is None:
            return {}
        key = (sem, i)
        if key in memo:
            return memo[key]
        memo[key] = {}
        out = {}
        inorder = not sem.startswith("Pool")
        rng = range(i + 1) if inorder else (i,)
        for j in rng:
            _, ins = proc_events[sem][j]
            for (s2, v2) in waits_of.get(id(ins), []):
                if out.get(s2, 0) < v2:
                    out[s2] = v2
                sub = holds(s2, v2, depth + 1)
                for s3, v3 in sub.items():
                    if out.get(s3, 0) < v3:
                        out[s3] = v3
        memo[key] = out
        return out

    own_tick = {}
    for sem, lst in proc_events.items():
        for tick, ins in lst:
            own_tick[(id(ins), sem)] = tick

    pruned = 0
    for ins in insts:
        si = getattr(ins, "sync_info", None)
        if si is None or not si.on_wait or len(si.on_wait) < 2:
            continue
        ow = list(si.on_wait)
        kept = list(ow)
        for w in ow:
            if len(kept) == 1:
                break
            mine = own_tick.get((id(ins), w.ant_name))
            if mine is not None and w.wait_value <= mine - 1:
                kept.remove(w)
                pruned += 1
                continue
            others = [o for o in kept if o is not w]
            for o in others:
                h = holds(o.ant_name, o.wait_value)
                if h.get(w.ant_name, 0) >= w.wait_value:
                    kept.remove(w)
                    pruned += 1
                    break
        si.on_wait = kept
    return pruned


def _host_prep(x, conv_w, centroids):
    from concourse import mybir
    bf16np = mybir.dt.np(mybir.dt.bfloat16)
    f8np = mybir.dt.np(mybir.dt.float8e4)

    x = np.ascontiguousarray(x, dtype=np.float32)
    L = H * W
    xf = x.reshape(C, L)
    norm = np.sqrt((xf.astype(np.float64) ** 2).sum(0))
    inv_norm = (1.0 / np.maximum(norm, 1e-12)).astype(np.float32)
    xn = xf * inv_norm[None, :]
    ii = np.arange(H, dtype=np.float32)
    mi = np.minimum(H - 1 - ii, ii)
    m_ = np.minimum(mi[:, None], mi[None, :]).astype(np.float32)
    m2 = m_ * m_

    # fp8 e4m3 streams carry xn * 64 (unit-norm values ~±0.2 are below the
    # fp8 normal range without the prescale); exp applies 1/64 on chip and
    # the uniform scales on the VLAD partials cancel in the normalizations.
    # The border mask^4 is split as mask^2 (folded into isum on chip) times
    # mask^2 (folded into the xlcn stream) so both stay in fp8 range.
    mask2 = m2.reshape(L)
    maxm2 = float(mask2.max())
    xn_pad = np.zeros((C, (H + 2) * W), np.float32)
    xn_pad[:, W:(H + 1) * W] = xn * 64.0
    m4_pad = np.zeros(((H + 2) * W,), np.float32)
    m4_pad[W:(H + 1) * W] = mask2 / maxm2

    # conv weights packed [128, CT*K] to share one DMA with shm
    cwtp = np.ascontiguousarray(
        conv_w.T.reshape(CT, 128, K).transpose(1, 0, 2).reshape(128, CT * K)
    ).astype(f8np)

    # banded shift matrices for the 3x3 box-sum over flattened L (W=192);
    # 6th matrix is zero (pad for the DoubleRow pair (2, 3))
    delta = np.array([-193, -192, -191, -1, 0, 1, 191, 192, 193])
    q = np.arange(128)
    shm = np.zeros((6, 128, 128), np.float32)               # [j+2, q, i]
    for jj in range(-2, 3):
        for d in delta:
            ivals = q - d + 128 * jj                        # i = q - (d - 128j)
            ok = (ivals >= 0) & (ivals < 128)
            shm[jj + 2, q[ok], ivals[ok]] = 1.0
    shm = np.ascontiguousarray(shm.transpose(1, 0, 2).reshape(128, 6 * 128)
                               ).astype(f8np)
    c8 = np.ascontiguousarray(np.concatenate([cwtp, shm], axis=1))

    in_maps = []
    for core in range(M):
        r0 = core * RPC
        sl = slice(r0 * W, (r0 + RPC + 2) * W)
        m4c = m4_pad[sl].copy()
        m4c[0:W] = 0.0
        m4c[(RPC + 1) * W:] = 0.0                # halo rows contribute 0
        m2c = m4_pad[sl]                         # xlcn-side mask^2 (scaled)
        xsc = np.empty((Ls, C + 1), np.float32)
        xsc[:, 0:C] = xn_pad[:, sl].T * m2c[:, None]
        xsc[:, C] = 64.0 * m2c                   # matches the xn*64 scale
        # pack so each of the 128 partitions is one contiguous DMA run
        xclp = np.ascontiguousarray(
            xn_pad[:, sl].reshape(CT, 128, Ls).transpose(1, 0, 2)
            .reshape(128, CT * Ls)).astype(f8np)
        xlcnp = np.ascontiguousarray(
            xsc.reshape(NT, 128, C + 1).transpose(1, 0, 2)
            .reshape(128, NT * (C + 1))).astype(f8np)
        in_maps.append({
            "xcl": xclp,
            "xlcn": xlcnp,
            "c8": c8,
            "m4": np.ascontiguousarray(m4c.reshape(NT, 128).T),
        })
    return in_maps


def _ensure_ntff_hook():
    """Install the axon NTFF profile hook if the image's antenv lacks it."""
    import types
    try:
        from antenv.axon_hooks import get_axon_ntff_profile_hook  # noqa: F401
        return
    except ImportError:
        pass
    if "/root/.axon_site" not in sys.path:
        sys.path.insert(0, "/root/.axon_site")
    from trn_agent_boot.trn_boot import _ntff_profile_via_ctypes
    hook = _ntff_profile_via_ctypes("/opt/axon/libaxon_pjrt.so")
    mod = types.ModuleType("antenv.axon_hooks")
    mod.get_axon_ntff_profile_hook = lambda: hook
    mod.set_axon_ntff_profile_hook = lambda h: None
    import antenv
    antenv.axon_hooks = mod
    sys.modules["antenv.axon_hooks"] = mod


def _install_neff_cache():
    """Cache compiled NEFFs across processes, keyed by BIR content hash."""
    import hashlib
    import shutil
    import concourse.bass2jax as b2j

    orig = b2j.compile_bir_kernel
    if getattr(orig, "_neff_cached", False):
        return

    def cached(bir_json, tmpdir, neff_name="file.neff"):
        h = hashlib.sha256(
            bir_json if isinstance(bir_json, bytes) else bir_json.encode()
        ).hexdigest()[:24]
        cdir = "/tmp/neff_cache"
        os.makedirs(cdir, exist_ok=True)
        cpath = os.path.join(cdir, h + ".neff")
        if os.path.exists(cpath):
            dst = os.path.join(tmpdir, neff_name)
            os.makedirs(tmpdir, exist_ok=True)
            shutil.copy(cpath, dst)
            return dst
        out = orig(bir_json, tmpdir, neff_name=neff_name)
        shutil.copy(out, cpath)
        return out

    cached._neff_cached = True
    b2j.compile_bir_kernel = cached


def kernel(x, conv_w, centroids):
    import concourse.bass_utils as bu
    from concourse.bass_utils import run_bass_kernel_spmd
    _install_neff_cache()
    if TRACE:
        _ensure_ntff_hook()
        bu.upload_artifacts = lambda tmpdir: "local://" + tmpdir

    if "nc" not in _CACHE:
        _CACHE["nc"] = _build_nc()
    nc = _CACHE["nc"]
    in_maps = _host_prep(np.asarray(x), np.asarray(conv_w), np.asarray(centroids))
    res = run_bass_kernel_spmd(nc, in_maps, list(range(M)), trace=TRACE)
    _CACHE["last"] = res
    red = np.zeros((K, C + 1), np.float32)
    for r in res.results:
        red += np.asarray(r["y"], dtype=np.float32)
    vlad = red[:, :C] - red[:, C:C + 1] * np.asarray(centroids, np.float32)
    vlad /= np.maximum(np.sqrt((vlad ** 2).sum(1))[:, None], 1e-12)
    v = vlad.reshape(1, K * C)
    v /= np.maximum(np.sqrt((v ** 2).sum()), 1e-12)
    return v.astype(np.float32)


# revision 80
# speedup vs baseline: 1.1021x; 1.1021x over previous
"""NetVLAD (vq_codebook) Trainium2 Bass kernel, 8-way spatially sharded.

Math (validated vs reference to ~1.6e-3 rel, tolerance 2e-2):
  xn = x / ||x||_C per location (HOST); logits per l-tile directly in
  [L,K] layout (stationary x-tile, moving conv weights, fp8 both);
  e = exp(logits/64) f32; top-2 keep via per-tile Max8 ([:,1] = 2nd max);
  cnt = 3x3 box-sum of keep as banded shift-matrix matmuls on the PE in
  fp8 DoubleRow mode (pairs of shift groups, overlapping rhs APs, zero
  guard tiles — no transposes, no unfold/fold); w2 = e * mask^2/sumexp *
  cnt in fp8; VLAD partials [K,C+1] = w2.T @ [xn*mask^2*64 | mask^2*64]
  in fp8 DoubleRow (two l-tiles per matmul), all-reduced on host.
  The border mask^4 is split mask^2 * mask^2 across w2 and the xlcn
  stream so both stay inside fp8 e4m3 range; all uniform scale factors
  (64, 1/max(mask^2)) cancel in the final L2 normalizations.

All HBM streams are fp8 (x ~5.2 MB/core total) packed host-side so each
SBUF partition is one contiguous DMA run; a 6-stage chunk pipeline
overlaps DMA / PE logits / Scalar exp / DVE top-2 / PE cnt / Pool ee /
DVE w2 / PE VLAD, with dummy-matmul sync absorbers keeping every
instruction at <= 1 sync wait (walrus codegen limit).

Sharding: H=192 rows split 8 ways (24 rows/core + 1 halo row each side).
Everything local per core; [K,C+1] partials reduced on host.
"""
import os
import sys

sys.path.insert(0, "/opt/trn_rl_repo")
os.environ.setdefault("MYCRO_LOCAL_CACHE", "1")

import numpy as np

C, H, W, K = 512, 192, 192, 64
M = 8                      # cores
RPC = H // M               # 24 rows per core
Ls = (RPC + 2) * W         # 4992 slab locations (incl. 1 halo row each side)
NT = Ls // 128             # 39 l-tiles
CT = C // 128              # 4 c-tiles
CH = 8                     # max tiles per pipeline chunk
# small final chunks shorten the serial dependency tail after the last
# DMA wave lands
CHUNK_SIZES = [6, 6, 6, 6, 6, 6, 3]
assert sum(CHUNK_SIZES) == NT
CHUNKS = []
_o = 0
for _s in CHUNK_SIZES:
    CHUNKS.append((_o, _o + _s))
    _o += _s

TRACE = False              # set by test.py for profiling runs
_CACHE = {}


def _build_nc():
    import concourse.bass as bass
    import concourse.tile as tile
    from concourse import mybir

    f32 = mybir.dt.float32
    bf16 = mybir.dt.bfloat16
    f8 = mybir.dt.float8e4
    AF = mybir.ActivationFunctionType
    OP = mybir.AluOpType
    AX = mybir.AxisListType

    nc = bass.Bass()
    # host-packed layouts: one contiguous run per partition -> wide DMA
    # descriptors (the [Ls, C+1] row layout was descriptor-bound in fp8)
    xcl = nc.dram_tensor("xcl", [128, CT * Ls], f8, kind="ExternalInput")
    xlcn = nc.dram_tensor("xlcn", [128, NT * (C + 1)], f8,
                          kind="ExternalInput")
    c8 = nc.dram_tensor("c8", [128, CT * K + 6 * 128], f8,
                        kind="ExternalInput")
    m4 = nc.dram_tensor("m4", [128, NT], f32, kind="ExternalInput")
    y = nc.dram_tensor("y", [K, C + 1], f32, kind="ExternalOutput")

    with tile.TileContext(nc) as tc:
        with tc.tile_pool(name="big", bufs=1) as big:
            xcl_sb = big.tile([128, CT * Ls], f8, tag="xcl")
            xlcn_sb = big.tile([128, NT * (C + 1)], f8, tag="xlcn")
            c8_sb = big.tile([128, CT * K + 6 * 128], f8, tag="c8")
            cwt_sb = c8_sb[:, 0:CT * K]
            shm_sb = c8_sb[:, CT * K:]
            m4_sb = big.tile([128, NT], f32, tag="m4")
            expb = big.tile([128, NT * K], f32, tag="expb")
            eeb = big.tile([128, NT * K], f32, tag="eeb")
            # 2 zero guard tiles on each side let the cnt shift matmuls run
            # full chunk width with uniform start/stop accumulation groups;
            # keep/w2 are fp8 so cnt and VLAD matmuls run in DoubleRow mode
            keeplk = big.tile([128, (NT + 5) * K], f8, tag="keeplk")
            w2b = big.tile([128, NT * K], f8, tag="w2b")
            mx8 = big.tile([128, NT * 8], f32, tag="mx8")
            sume = big.tile([128, NT], f32, tag="sume")
            isume = big.tile([128, NT], f32, tag="isume")
            isume2 = big.tile([128, NT], f32, tag="isume2")
            vl_sb = big.tile([K, C + 1], f32, tag="vl")
            zbias = big.tile([128, 1], f32, tag="zbias")
            zscr = big.tile([128, 1], f32, tag="zscr")
            nc.vector.memset(zbias[:], 0.0)
            # scalar-engine touch: absorbs the memset completion so the first
            # exp activation carries only its PE wait
            nc.scalar.copy(zscr[:], zbias[:])

            # input DMAs: small first, then xcl chunks interleaved with the
            # xlcn waves (lag 2) so VLAD waves can start mid-kernel
            xc3 = xcl[:].rearrange("p (ct l) -> p ct l", l=Ls)
            xs3 = xcl_sb[:].rearrange("p (ct l) -> p ct l", l=Ls)

            # Each dma_start costs ~0.65us of SP sequencer time, so waves are
            # coarse (11 issues total) and ordered by need: first xcl wave,
            # consts, rest of xcl (compute-gating), then xlcn.
            # (Issuing from other sequencers regressed: the Tile scheduler
            # interleaves those issues with that engine's compute work.)
            XCLW = list(CHUNKS)
            XLCNW = list(CHUNKS)

            def dma_xcl_w(t0, t1):
                nc.sync.dma_start(
                    xs3[:, :, t0 * 128:t1 * 128],
                    xc3[:, :, t0 * 128:t1 * 128],
                )

            def dma_xlcn_w(t0, t1):
                nc.sync.dma_start(
                    xlcn_sb[:, t0 * (C + 1):t1 * (C + 1)],
                    xlcn[:, t0 * (C + 1):t1 * (C + 1)],
                )

            dma_xcl_w(*XCLW[0])
            nc.sync.dma_start(c8_sb[:], c8[:])
            nc.sync.dma_start(m4_sb[:], m4[:])
            for w in XCLW[1:]:
                dma_xcl_w(*w)
            for w in XLCNW:
                dma_xlcn_w(*w)

            e3f = expb[:].rearrange("p (t k) -> p t k", k=K)
            ee3f = eeb[:].rearrange("p (t k) -> p t k", k=K)
            k3f = keeplk[:].rearrange("p (t k) -> p t k", k=K)  # guard-offset
            w23f = w2b[:].rearrange("p (t k) -> p t k", k=K)
            mx3f = mx8[:].rearrange("p (t e) -> p t e", e=8)
            # Pool touch: absorbs the m4 DMA completion so the per-chunk
            # mask multiplies carry only their DVE(recip) wait
            nc.gpsimd.tensor_copy(isume2[:, 0:1], m4_sb[:, 0:1])
            nc.vector.memset(keeplk[:, 0:2 * K], 0.0)
            nc.vector.memset(keeplk[:, (NT + 2) * K:(NT + 5) * K], 0.0)

            with tc.tile_pool(name="pp", bufs=1, space="PSUM") as pp:
                pv0 = pp.tile([K, C], f32, tag="pv0", bufs=1)
                pv1 = pp.tile([K, 1], f32, tag="pv1", bufs=1)
                pcnts = {}

                # two dummies absorb the c8 DMA wait (cwt+shm views); a long
                # HAM warm-up burst is not worth delaying logits(0) — the
                # DVE chain that paces the kernel starts after exp(chunk 0)
                for i in range(2):
                    src = shm_sb if i == 1 else cwt_sb
                    dmy = pp.tile([128, K], f32, tag="dmy", bufs=2)
                    nc.tensor.matmul(dmy[0:64, 0:64], lhsT=src[:, 0:64],
                                     rhs=src[:, 0:64], start=True, stop=True)

                def emit_logits(ci):
                    t0, t1 = CHUNKS[ci]
                    # dummy absorbs this chunk's xcl DMA completion so real
                    # matmuls carry only the psum-rotation wait
                    dmy = pp.tile([128, K], f32, tag="dmy", bufs=2)
                    nc.tensor.matmul(
                        dmy[0:64, 0:64],
                        lhsT=xcl_sb[:, t0 * 128:t0 * 128 + 64],
                        rhs=xcl_sb[:, t0 * 128:t0 * 128 + 64],
                        start=True, stop=True)
                    plog = pp.tile([128, CH * K], f32, tag="plog", bufs=2)
                    for t in range(t0, t1):
                        for ct in range(CT):
                            nc.tensor.matmul(
                                plog[:, (t - t0) * K:(t - t0 + 1) * K],
                                lhsT=xcl_sb[:, ct * Ls + t * 128:
                                            ct * Ls + (t + 1) * 128],
                                rhs=cwt_sb[:, ct * K:(ct + 1) * K],
                                start=(ct == 0), stop=(ct == CT - 1))
                    return plog

                def emit_exp(ci, plog):
                    # scale 1/64 undoes the fp8-range prescale of xcl
                    t0, t1 = CHUNKS[ci]
                    nc.scalar.activation(
                        expb[:, t0 * K:t1 * K], plog[:, 0:(t1 - t0) * K],
                        AF.Exp, bias=zbias[:, 0:1], scale=1.0 / 64.0)

                def emit_dve(ci):
                    t0, t1 = CHUNKS[ci]
                    ch = t1 - t0
                    e3 = e3f[:, t0:t1]
                    k3 = k3f[:, t0:t1]
                    # keep (gating the PE cnt matmuls) first: top-8 per tile
                    # ([:,0]=max, [:,1]=second max), then the compare
                    for t in range(t0, t1):
                        nc.vector.max(mx8[:, t * 8:(t + 1) * 8],
                                      expb[:, t * K:(t + 1) * K])
                    m2bc = mx3f[:, t0:t1, 1:2].broadcast_to([128, ch, K])
                    nc.vector.tensor_tensor(k3f[:, t0 + 2:t1 + 2], e3, m2bc,
                                            op=OP.is_ge)
                    # softmax normalizer (gates only the Pool ee multiply)
                    nc.vector.tensor_reduce(sume[:, t0:t1], e3, axis=AX.X,
                                            op=OP.add)
                    nc.vector.reciprocal(isume[:, t0:t1], sume[:, t0:t1])

                def emit_pool_ee(ci):
                    # the mask^2 fold runs on Pool too (keeps it off the
                    # pacing Vector engine); isume2 is Pool-local
                    t0, t1 = CHUNKS[ci]
                    ch = t1 - t0
                    nc.gpsimd.tensor_tensor(isume2[:, t0:t1],
                                            isume[:, t0:t1],
                                            m4_sb[:, t0:t1], op=OP.mult)
                    ibc = isume2[:, t0:t1][:, :, None].broadcast_to(
                        [128, ch, K])
                    nc.gpsimd.tensor_tensor(ee3f[:, t0:t1], e3f[:, t0:t1],
                                            ibc, op=OP.mult)

                def emit_cnt(ci):
                    # full-chunk-width DoubleRow matmuls: each contracts a
                    # pair of shift groups (j, j+1); the 6th shift matrix is
                    # zero. Zero guard tiles cover out-of-slab sources. The
                    # pair rhs is an overlapping AP (same keep data shifted
                    # by one tile).
                    t0, t1 = CHUNKS[ci]
                    w = (t1 - t0) * K
                    pcnt = pp.tile([128, CH * K], f32, tag="pcnt", bufs=2)
                    pcnts[ci] = pcnt
                    for pi, j in enumerate((-2, 0, 2)):
                        lhsT = shm_sb[:, (j + 2) * 128:(j + 4) * 128]\
                            .rearrange("p (two i) -> p two i", two=2)
                        base = keeplk[:, (t0 + j + 2) * K:(t1 + j + 2) * K]
                        rhs = type(base)(base.tensor, base.offset,
                                         [list(base.ap[0]), [K, 2], [1, w]])
                        nc.tensor.matmul(
                            pcnt[:, 0:w], lhsT=lhsT, rhs=rhs,
                            perf_mode=mybir.MatmulPerfMode.DoubleRow,
                            start=(pi == 0), stop=(pi == 2))

                def emit_pool_w2(ci):
                    # on DVE: GPSIMD cannot read PSUM (pcnt). The touch copy
                    # absorbs the Pool(ee) completion so the TT carries only
                    # the PE(pcnt) wait (one sync wait per instruction).
                    t0, t1 = CHUNKS[ci]
                    pc3 = pcnts[ci][:, 0:(t1 - t0) * K].rearrange(
                        "p (t k) -> p t k", k=K)
                    nc.vector.tensor_copy(w2b[:, t0 * K:t0 * K + 1],
                                          eeb[:, t0 * K:t0 * K + 1])
                    nc.vector.tensor_tensor(w23f[:, t0:t1], ee3f[:, t0:t1],
                                            pc3, op=OP.mult)

                def emit_vlad(ci):
                    t0, t1 = CHUNKS[ci]
                    # dummy absorbs the xlcn wave DMA completion
                    dmy = pp.tile([128, K], f32, tag="dmy", bufs=2)
                    nc.tensor.matmul(
                        dmy[0:64, 0:64],
                        lhsT=xlcn_sb[:, t0 * (C + 1):t0 * (C + 1) + 64],
                        rhs=xlcn_sb[:, t0 * (C + 1):t0 * (C + 1) + 64],
                        start=True, stop=True)
                    xl3 = xlcn_sb[:].rearrange("p (t c) -> p t c", c=C + 1)
                    t = t0
                    while t < t1:
                        if t + 1 < t1:   # DoubleRow pair of l-tiles
                            lt = w2b[:, t * K:(t + 2) * K].rearrange(
                                "p (two k) -> p two k", two=2)
                            nc.tensor.matmul(
                                pv0[:], lhsT=lt, rhs=xl3[:, t:t + 2, 0:C],
                                perf_mode=mybir.MatmulPerfMode.DoubleRow,
                                start=(t == 0), stop=(t + 2 == NT))
                            nc.tensor.matmul(
                                pv1[:], lhsT=lt, rhs=xl3[:, t:t + 2, C:C + 1],
                                perf_mode=mybir.MatmulPerfMode.DoubleRow,
                                start=(t == 0), stop=(t + 2 == NT))
                            t += 2
                        else:            # odd tail tile
                            lt = w2b[:, t * K:(t + 1) * K]
                            nc.tensor.matmul(
                                pv0[:], lhsT=lt, rhs=xl3[:, t, 0:C],
                                start=(t == 0), stop=(t + 1 == NT))
                            nc.tensor.matmul(
                                pv1[:], lhsT=lt, rhs=xl3[:, t, C:C + 1],
                                start=(t == 0), stop=(t + 1 == NT))
                            t += 1

                # VLAD runs with lag 2 and is emitted BEFORE cnt: its inputs
                # (w2 of two chunks back + an old xlcn wave) are long ready,
                # so it fills the PE stall while the DVE finishes keep(ci),
                # keeping the PE busy (and its HAM clock warm)
                for ci in range(len(CHUNKS)):
                    plog = emit_logits(ci)
                    emit_exp(ci, plog)
                    emit_dve(ci)
                    emit_pool_ee(ci)
                    if ci >= 2:
                        emit_vlad(ci - 2)
                    if ci >= 1:
                        emit_cnt(ci - 1)
                        emit_pool_w2(ci - 1)
                last = len(CHUNKS) - 1
                emit_vlad(last - 1)
                emit_cnt(last)
                emit_pool_w2(last)
                emit_vlad(last)

                nc.scalar.copy(vl_sb[:, 0:C], pv0[:])
                nc.scalar.copy(vl_sb[:, C:C + 1], pv1[:])
                nc.sync.dma_start(y[:], vl_sb[:])
    _prune_waits(nc)
    return nc


def _prune_waits(nc):
    """Drop semaphore waits that are transitively implied by another wait on
    the same instruction (see kernel_baseline.py for the full rationale)."""
    insts = [ins for bb in nc.main_func.blocks for ins in bb.instructions]
    proc_events = {}
    waits_of = {}
    carried = {}   # engine -> waits of non-updating instrs (e.g. Ldweights)
    for ins in insts:
        si = getattr(ins, "sync_info", None)
        if si is None:
            continue
        ow = list(si.on_wait or [])
        waits_of[id(ins)] = [(w.ant_name, w.wait_value) for w in ow]
        ups = [u for u in (si.on_update or [])
               if getattr(u, "update_mode", None) in ("sem-inc", "sem-add-imm")]
        eng = getattr(ins, "engine", None)
        if not ups:
            # a waiting-but-not-updating instruction (Ldweights): its waits
            # are guaranteed held once the NEXT updating instruction on the
            # same engine ticks (in-order issue; LDW completes before its MM)
            if ow and eng is not None:
                carried.setdefault(eng, []).extend(waits_of[id(ins)])
            continue
        if eng in carried and carried[eng]:
            waits_of[id(ins)] = waits_of[id(ins)] + carried.pop(eng)
        for u in ups:
            lst = proc_events.setdefault(u.ant_name, [])
            prev = lst[-1][0] if lst else 0
            lst.append((prev + (u.update_value or 1), ins))

    import bisect

    def prefix_index(sem, v):
        lst = proc_events.get(sem)
        if not lst:
            return None
        ticks = [t for t, _ in lst]
        i = bisect.bisect_left(ticks, v)
        return i if i < len(lst) else None

    memo = {}

    def holds(sem, v, depth=0):
        if depth > 6:
            return {}
        i = prefix_index(sem, v)
        if i is None:
            return {}
        key = (sem, i)
        if key in memo:
            return memo[key]
        memo[key] = {}
        out = {}
        inorder = not sem.startswith("Pool")
        rng = range(i + 1) if inorder else (i,)
        for j in rng:
            _, ins = proc_events[sem][j]
            for (s2, v2) in waits_of.get(id(ins), []):
                if out.get(s2, 0) < v2:
                    out[s2] = v2
                sub = holds(s2, v2, depth + 1)
                for s3, v3 in sub.items():
                    if out.get(s3, 0) < v3:
                        out[s3] = v3
        memo[key] = out
        return out

    own_tick = {}
    for sem, lst in proc_events.items():
        for tick, ins in lst:
            own_tick[(id(ins), sem)] = tick

    pruned = 0
    for ins in insts:
        si = getattr(ins, "sync_info", None)
        if si is None or not si.on_wait or len(si.on_wait) < 2:
            continue
        ow = list(si.on_wait)
        kept = list(ow)
        for w in ow:
            if len(kept) == 1:
                break
            mine = own_tick.get((id(ins), w.ant_name))
            if mine is not None and w.wait_value <= mine - 1:
                kept.remove(w)
                pruned += 1
                continue
            others = [o for o in kept if o is not w]
            for o in others:
                h = holds(o.ant_name, o.wait_value)
                if h.get(w.ant_name, 0) >= w.wait_value:
                    kept.remove(w)
                    pruned += 1
                    break
        si.on_wait = kept
    return pruned


def _host_prep(x, conv_w, centroids):
    from concourse import mybir
    bf16np = mybir.dt.np(mybir.dt.bfloat16)
    f8np = mybir.dt.np(mybir.dt.float8e4)

    x = np.ascontiguousarray(x, dtype=np.float32)
    L = H * W
    xf = x.reshape(C, L)
    norm = np.sqrt((xf.astype(np.float64) ** 2).sum(0))
    inv_norm = (1.0 / np.maximum(norm, 1e-12)).astype(np.float32)
    xn = xf * inv_norm[None, :]
    ii = np.arange(H, dtype=np.float32)
    mi = np.minimum(H - 1 - ii, ii)
    m_ = np.minimum(mi[:, None], mi[None, :]).astype(np.float32)
    m2 = m_ * m_

    # fp8 e4m3 streams carry xn * 64 (unit-norm values ~±0.2 are below the
    # fp8 normal range without the prescale); exp applies 1/64 on chip and
    # the uniform scales on the VLAD partials cancel in the normalizations.
    # The border mask^4 is split as mask^2 (folded into isum on chip) times
    # mask^2 (folded into the xlcn stream) so both stay in fp8 range.
    mask2 = m2.reshape(L)
    maxm2 = float(mask2.max())
    xn_pad = np.zeros((C, (H + 2) * W), np.float32)
    xn_pad[:, W:(H + 1) * W] = xn * 64.0
    m4_pad = np.zeros(((H + 2) * W,), np.float32)
    m4_pad[W:(H + 1) * W] = mask2 / maxm2

    # conv weights packed [128, CT*K] to share one DMA with shm
    cwtp = np.ascontiguousarray(
        conv_w.T.reshape(CT, 128, K).transpose(1, 0, 2).reshape(128, CT * K)
    ).astype(f8np)

    # banded shift matrices for the 3x3 box-sum over flattened L (W=192);
    # 6th matrix is zero (pad for the DoubleRow pair (2, 3))
    delta = np.array([-193, -192, -191, -1, 0, 1, 191, 192, 193])
    q = np.arange(128)
    shm = np.zeros((6, 128, 128), np.float32)               # [j+2, q, i]
    for jj in range(-2, 3):
        for d in delta:
            ivals = q - d + 128 * jj                        # i = q - (d - 128j)
            ok = (ivals >= 0) & (ivals < 128)
            shm[jj + 2, q[ok], ivals[ok]] = 1.0
    shm = np.ascontiguousarray(shm.transpose(1, 0, 2).reshape(128, 6 * 128)
                               ).astype(f8np)
    c8 = np.ascontiguousarray(np.concatenate([cwtp, shm], axis=1))

    in_maps = []
    for core in range(M):
        r0 = core * RPC
        sl = slice(r0 * W, (r0 + RPC + 2) * W)
        m4c = m4_pad[sl].copy()
        m4c[0:W] = 0.0
        m4c[(RPC + 1) * W:] = 0.0                # halo rows contribute 0
        m2c = m4_pad[sl]                         # xlcn-side mask^2 (scaled)
        xsc = np.empty((Ls, C + 1), np.float32)
        xsc[:, 0:C] = xn_pad[:, sl].T * m2c[:, None]
        xsc[:, C] = 64.0 * m2c                   # matches the xn*64 scale
        # pack so each of the 128 partitions is one contiguous DMA run
        xclp = np.ascontiguousarray(
            xn_pad[:, sl].reshape(CT, 128, Ls).transpose(1, 0, 2)
            .reshape(128, CT * Ls)).astype(f8np)
        xlcnp = np.ascontiguousarray(
            xsc.reshape(NT, 128, C + 1).transpose(1, 0, 2)
            .reshape(128, NT * (C + 1))).astype(f8np)
        in_maps.append({
            "xcl": xclp,
            "xlcn": xlcnp,
            "c8": c8,
            "m4": np.ascontiguousarray(m4c.reshape(NT, 128).T),
        })
    return in_maps


def _ensure_ntff_hook():
    """Install the axon NTFF profile hook if the image's antenv lacks it."""
    import types
    try:
        from antenv.axon_hooks import get_axon_ntff_profile_hook  # noqa: F401
        return
    except ImportError:
        pass
    if "/root/.axon_site" not in sys.path:
        sys.path.insert(0, "/root/.axon_site")
    from trn_agent_boot.trn_boot import _ntff_profile_via_ctypes
    hook = _ntff_profile_via_ctypes("/opt/axon/libaxon_pjrt.so")
    mod = types.ModuleType("antenv.axon_hooks")
    mod.get_axon_ntff_profile_hook = lambda: hook
    mod.set_axon_ntff_profile_hook = lambda h: None
    import antenv
    antenv.axon_hooks = mod
    sys.modules["antenv.axon_hooks"] = mod


def _install_neff_cache():
    """Cache compiled NEFFs across processes, keyed by BIR content hash."""
    import hashlib
    import shutil
    import concourse.bass2jax as b2j

    orig = b2j.compile_bir_kernel
    if getattr(orig, "_neff_cached", False):
        return

    def cached(bir_json, tmpdir, neff_name="file.neff"):
        h = hashlib.sha256(
            bir_json if isinstance(bir_json, bytes) else bir_json.encode()
        ).hexdigest()[:24]
        cdir = "/tmp/neff_cache"
        os.makedirs(cdir, exist_ok=True)
        cpath = os.path.join(cdir, h + ".neff")
        if os.path.exists(cpath):
            dst = os.path.join(tmpdir, neff_name)
            os.makedirs(tmpdir, exist_ok=True)
            shutil.copy(cpath, dst)
            return dst
        out = orig(bir_json, tmpdir, neff_name=neff_name)
        shutil.copy(out, cpath)
        return out

    cached._neff_cached = True
    b2j.compile_bir_kernel = cached


def kernel(x, conv_w, centroids):
    import concourse.bass_utils as bu
    from concourse.bass_utils import run_bass_kernel_spmd
    _install_neff_cache()
    if TRACE:
        _ensure_ntff_hook()
        bu.upload_artifacts = lambda tmpdir: "local://" + tmpdir

    if "nc" not in _CACHE:
        _CACHE["nc"] = _build_nc()
    nc = _CACHE["nc"]
    in_maps = _host_prep(np.asarray(x), np.asarray(conv_w), np.asarray(centroids))
    res = run_bass_kernel_spmd(nc, in_maps, list(range(M)), trace=TRACE)
    _CACHE["last"] = res
    red = np.zeros((K, C + 1), np.float32)
    for r in res.results:
        red += np.asarray(r["y"], dtype=np.float32)
    vlad = red[:, :C] - red[:, C:C + 1] * np.asarray(centroids, np.float32)
    vlad /= np.maximum(np.sqrt((vlad ** 2).sum(1))[:, None], 1e-12)
    v = vlad.reshape(1, K * C)
    v /= np.maximum(np.sqrt((v ** 2).sum()), 1e-12)
    return v.astype(np.float32)
